# revision 9
# baseline (speedup 1.0000x reference)
"""Causal multi-head attention Trainium2 kernel (8 NeuronCores).

Problem: B=4, L=2048, D=1024, 16 heads x (dh=64, dv=64), causal mask.
Sharding: data-parallel over batch (4) x tensor-parallel over heads (2 groups
of 8). Core c handles batch c//2, head-group c%2. Each core computes its
partial output projection (ctx_g @ Wo_g); the host sums the two head-group
partials per batch and adds the bias.

v4: all-bf16 datapath. x transposed via plain bf16 matmuls against identity
(pipelines ~82ns/tile). S^T = K@Q^T for the two heads of a pair issued
back-to-back into disjoint PE row groups (partitions 0-63/64-127) so they run
concurrently. Attention pipelined at single-k-tile granularity: psc
[128,2,512] (2 PSUM banks) double-buffered decouples the S->exp->PV chain
(v3's 4-bank group was single-buffered and serialized S behind exp). exp is
width-restricted on every diagonal k-tile. Q/K projections for head-pair 0
interleave with the x-transposes; V l-tiles 6-15 and the next pair's Q/K
dribble between attention slots; the output projection for l-chunk j dribbles
into head-pair 3's attention of chunk j+1. Weight DMAs ordered so x loads
first; wo loads during attention.
"""

import numpy as np
from contextlib import ExitStack

import concourse.bass as bass
import concourse.tile as tile
from concourse import bacc, mybir
from concourse.masks import make_identity

F32 = mybir.dt.float32
BF16 = mybir.dt.bfloat16
AF = mybir.ActivationFunctionType

B, L, D = 4, 2048, 1024
N_HEAD, DH, DV = 16, 64, 64
N_CORES = 8
HPC = N_HEAD // 2          # heads per core (8)
OC = HPC * DH              # per-core projection width (512)
NHP = HPC // 2             # head-pairs per core (4)
NCH = L // 512             # q-chunks (4)
NLT = L // 128             # l-tiles (16)


class Emitter:
    """Projection work (optional V l-tile jobs + one head-pair's Q^T/K^T)
    emitted in 8-matmul units so it interleaves with attention."""

    def __init__(self, nc, hp, qkp, wp, psP, xt, wq, wk, vjobs=()):
        self.nc, self.psP, self.xt = nc, psP, xt
        self.units = list(vjobs)           # V jobs first: needed earliest
        self.wq_sb = wp.tile([128, 8, 128], BF16, tag="wq")
        self.wk_sb = wp.tile([128, 8, 128], BF16, tag="wk")
        nc.sync.dma_start(
            out=self.wq_sb,
            in_=wq[:, hp * 128:(hp + 1) * 128].rearrange("(t p) o -> p t o", p=128))
        nc.sync.dma_start(
            out=self.wk_sb,
            in_=wk[:, hp * 128:(hp + 1) * 128].rearrange("(t p) o -> p t o", p=128))
        self.qt = qkp.tile([128, L], BF16, tag="qt")
        self.kt = qkp.tile([128, L], BF16, tag="kt")
        for c in range(NCH):               # chunk-major for prologue overlap
            for w_sb, dst in ((self.wq_sb, self.qt), (self.wk_sb, self.kt)):
                self.units.append((w_sb, dst, c))

    def step(self):
        if not self.units:
            return False
        u = self.units.pop(0)
        if callable(u):
            u()
            return True
        w_sb, dst, c = u
        nc = self.nc
        pp = self.psP.tile([128, 512], F32, tag="pp")
        for d in range(8):
            nc.tensor.matmul(pp, w_sb[:, d, :],
                             self.xt[:, d, c * 512:(c + 1) * 512],
                             start=(d == 0), stop=(d == 7))
        nc.vector.tensor_copy(dst[:, c * 512:(c + 1) * 512], pp)
        return True

    def drain(self):
        while self.step():
            pass


def build_nc(l=L):
    assert l % 512 == 0
    nch = l // 512
    nlt = l // 128
    nc = bacc.Bacc("TRN2", target_bir_lowering=False, debug=False,
                   num_devices=N_CORES)

    x = nc.dram_tensor("x", [l, D], BF16, kind="ExternalInput").ap()
    wq = nc.dram_tensor("wq", [D, OC], BF16, kind="ExternalInput").ap()
    wk = nc.dram_tensor("wk", [D, OC], BF16, kind="ExternalInput").ap()
    wv = nc.dram_tensor("wv", [D, OC], BF16, kind="ExternalInput").ap()
    wo = nc.dram_tensor("wo", [OC, D], BF16, kind="ExternalInput").ap()
    out = nc.dram_tensor("out", [l, D], F32, kind="ExternalOutput").ap()

    with tile.TileContext(nc) as tc, ExitStack() as ctx:
        top = ctx.enter_context(tc.tile_pool(name="top", bufs=1))
        psP = ctx.enter_context(tc.tile_pool(name="psP", bufs=2, space="PSUM"))
        qkp = ctx.enter_context(tc.tile_pool(name="qkp", bufs=2))
        wp = ctx.enter_context(tc.tile_pool(name="wp", bufs=2))

        # V: [128(l), ltile, head, 65] - col 64 is ones (softmax denominator)
        vt = top.tile([128, nlt, HPC, DV + 1], BF16)
        ct = top.tile([128, NHP, l], BF16)        # normalized ctx^T
        xt = top.tile([128, 8, l], BF16)          # x^T, d-major
        wv_sb = top.tile([128, 8, OC], BF16)
        wo_sb = top.tile([128, NHP, D], BF16)
        trilf = top.tile([128, 128], F32)
        tril = top.tile([128, 128], BF16)
        identf = top.tile([128, 128], F32)
        ident = top.tile([128, 128], BF16)
        ones = top.tile([128, 1], BF16)

        make_identity(nc, identf)
        nc.vector.tensor_copy(ident, identf)
        nc.vector.memset(ones, 1.0)
        nc.vector.tensor_copy(
            vt[:, :, :, DV:DV + 1].rearrange("p t h c -> p (t h) c"),
            ones.broadcast_to((128, nlt * HPC, 1)))
        # causal keep-mask for S^T diag blocks: tril[k, q] = 1.0 iff q >= k
        nc.gpsimd.memset(trilf, 0.0)
        nc.gpsimd.affine_select(
            out=trilf, in_=trilf, compare_op=mybir.AluOpType.is_gt,
            fill=1.0, base=0, pattern=[[-1, 128]], channel_multiplier=1)
        nc.vector.tensor_copy(tril, trilf)

        def v_unit(lt):
            pp = psP.tile([128, OC], F32, tag="pp")
            for d in range(8):
                nc.tensor.matmul(pp, xt[:, d, lt * 128:(lt + 1) * 128],
                                 wv_sb[:, d, :], start=(d == 0), stop=(d == 7))
            nc.vector.tensor_copy(
                vt[:, lt, :, 0:DV],
                pp.rearrange("p (h v) -> p h v", h=HPC))

        # ---- Prologue: transpose x || QK(hp0), then wv + V(lt 0-5) --------
        with tc.tile_pool(name="pro", bufs=2) as pro, \
             tc.tile_pool(name="psT", bufs=3, space="PSUM") as psT:
            em = None
            for c in range(nch):
                xst = pro.tile([128, 4, D], BF16, tag="xst")
                nc.sync.dma_start(
                    out=xst,
                    in_=x[c * 512:(c + 1) * 512, :].rearrange(
                        "(s p) d -> p s d", p=128))
                if c == 0:      # x chunk 0 queued first, then wq/wk
                    em = Emitter(nc, 0, qkp, wp, psP, xt, wq, wk)
                if c == 1:      # wv only needed at prologue end (V units)
                    nc.sync.dma_start(
                        out=wv_sb, in_=wv.rearrange("(t p) o -> p t o", p=128))
                for d in range(8):
                    pt = psT.tile([128, 512], F32, tag="pt")
                    for s in range(4):
                        nc.tensor.matmul(
                            pt[:, s * 128:(s + 1) * 128],
                            xst[:, s, d * 128:(d + 1) * 128], ident,
                            start=True, stop=True)
                    nc.vector.tensor_copy(xt[:, d, c * 512:(c + 1) * 512], pt)
                em.step()       # Q proj chunk c
                em.step()       # K proj chunk c
            for lt in range(6):
                v_unit(lt)

        # ---- Main: attention + dribbled projections -----------------------
        with tc.tile_pool(name="phb", bufs=2) as phb, \
             tc.tile_pool(name="psS", bufs=2, space="PSUM") as psS, \
             tc.tile_pool(name="psC", bufs=1, space="PSUM") as psC:

            def o_unit(lt, n):
                pp = psP.tile([128, 512], F32, tag="pp")
                for v in range(NHP):
                    nc.tensor.matmul(pp, ct[:, v, lt * 128:(lt + 1) * 128],
                                     wo_sb[:, v, n * 512:(n + 1) * 512],
                                     start=(v == 0), stop=(v == NHP - 1))
                ost = ost_tiles[lt % 3]
                nc.vector.tensor_copy(ost[:, n * 512:(n + 1) * 512], pp)
                if n == 1:
                    nc.sync.dma_start(
                        out=out[lt * 128:(lt + 1) * 128, :], in_=ost)

            ost_tiles = [top.tile([128, D], F32, name=f"ost{i}")
                         for i in range(3)]
            oq = []             # output-projection dribble queue

            for hp in range(NHP):
                qt, kt = em.qt, em.kt
                vjobs = [(lambda lt=lt: v_unit(lt)) for lt in range(6, 16)] \
                    if hp == 0 else ()
                em = (Emitter(nc, hp + 1, qkp, wp, psP, xt, wq, wk, vjobs)
                      if hp + 1 < NHP else None)
                if hp == NHP - 1:   # wo needed from hp3's j0 end onward
                    nc.sync.dma_start(
                        out=wo_sb, in_=wo.rearrange("(t p) o -> p t o", p=128))
                n_slots = sum(4 * (j + 1) for j in range(nch))
                n_units = len(em.units) if em else 1
                cadence = max(1, n_slots // n_units)
                slot = 0
                for j in range(nch):
                    nkt = 4 * (j + 1)
                    pctx = [psC.tile([DV + 1, 512], F32, tag=f"pctx{h}",
                                     name=f"pctx{h}")
                            for h in range(2)]
                    prev = None
                    for g in range(nkt + 1):
                        pexp = None
                        c0 = 0
                        if g < nkt:
                            kt_i = g
                            c0 = max(0, kt_i - 4 * j) * 128
                            psc = psS.tile([128, 2, 512], F32, tag="psc")
                            for h in range(2):
                                nc.tensor.matmul(
                                    psc[:, h, c0:512],
                                    kt[64 * h:64 * h + 64,
                                       kt_i * 128:(kt_i + 1) * 128],
                                    qt[64 * h:64 * h + 64,
                                       j * 512 + c0:(j + 1) * 512],
                                    start=True, stop=True)
                            pexp = phb.tile([128, 2, 512], BF16, tag="pexp",
                                            bufs=3)
                            nc.scalar.activation(pexp[:, :, c0:512],
                                                 psc[:, :, c0:512],
                                                 AF.Exp, scale=0.125)
                            if kt_i - 4 * j >= 0:   # diag: staircase mask
                                for h in range(2):
                                    nc.vector.tensor_mul(
                                        pexp[:, h, c0:c0 + 128],
                                        pexp[:, h, c0:c0 + 128], tril)
                        if prev is not None:
                            pkt, ppexp, pc0 = prev
                            for h in range(2):
                                nc.tensor.matmul(
                                    pctx[h][:, pc0:512],
                                    vt[:, pkt, 2 * hp + h, :],
                                    ppexp[:, h, pc0:512],
                                    start=(pkt == 0),
                                    stop=(pkt == 4 * j + 3))
                            if em and slot % cadence == 0:
                                em.step()
                            elif oq:
                                oq.pop(0)()
                            slot += 1
                        prev = (g, pexp, c0) if g < nkt else None
                    for h in range(2):
                        rs = phb.tile([1, 512], F32, tag="rs")
                        nc.vector.tensor_copy(rs, pctx[h][DV:DV + 1, :])
                        inv = phb.tile([1, 512], F32, tag="inv")
                        nc.vector.reciprocal_approx_fast(out=inv, in_=rs)
                        bc = phb.tile([64, 512], F32, tag="bc")
                        nc.gpsimd.partition_broadcast(out_ap=bc, in_ap=inv)
                        nc.vector.tensor_mul(
                            ct[64 * h:64 * h + 64, hp, j * 512:(j + 1) * 512],
                            pctx[h][0:DV, :], bc)
                    if hp == NHP - 1:   # ct chunk j complete: queue O-proj
                        for lt in range(4 * j, 4 * j + 4):
                            for n in range(2):
                                oq.append(lambda lt=lt, n=n: o_unit(lt, n))
                if em:
                    em.drain()
            while oq:
                oq.pop(0)()

    nc.compile()
    return nc


def make_in_maps(x, Wq, Wk, Wv, Wo):
    import ml_dtypes
    BF = ml_dtypes.bfloat16
    xb = [np.ascontiguousarray(x[b]).astype(BF) for b in range(B)]
    in_maps = []
    for c in range(N_CORES):
        b, g = c // 2, c % 2
        in_maps.append({
            "x": xb[b],
            "wq": np.ascontiguousarray(Wq[:, g * OC:(g + 1) * OC]).astype(BF),
            "wk": np.ascontiguousarray(Wk[:, g * OC:(g + 1) * OC]).astype(BF),
            "wv": np.ascontiguousarray(Wv[:, g * OC:(g + 1) * OC]).astype(BF),
            "wo": np.ascontiguousarray(Wo[g * OC:(g + 1) * OC, :]).astype(BF),
        })
    return in_maps


_NC_CACHE = {}


def _get_nc():
    if "nc" not in _NC_CACHE:
        _NC_CACHE["nc"] = build_nc()
    return _NC_CACHE["nc"]


def _numpy_fallback(x, Wq, Wk, Wv, Wo, bo, mask):
    Bsz, Lq, _ = x.shape
    Q = (x @ Wq).reshape(Bsz, Lq, N_HEAD, DH).transpose(0, 2, 1, 3)
    K = (x @ Wk).reshape(Bsz, Lq, N_HEAD, DH).transpose(0, 2, 1, 3)
    V = (x @ Wv).reshape(Bsz, Lq, N_HEAD, DV).transpose(0, 2, 1, 3)
    s = np.einsum("bhqd,bhkd->bhqk", Q, K) / np.sqrt(np.float32(DH))
    s = np.where(mask, s, -np.inf)
    s = s - s.max(axis=-1, keepdims=True)
    p = np.exp(s)
    p /= p.sum(axis=-1, keepdims=True)
    ctxv = np.einsum("bhqk,bhkv->bhqv", p, V)
    ctxv = ctxv.transpose(0, 2, 1, 3).reshape(Bsz, Lq, N_HEAD * DV)
    return (ctxv @ Wo + bo).astype(np.float32)


def run_on_hw(in_maps, trace=False):
    from concourse.bass_utils import run_bass_kernel_spmd
    nc = _get_nc()
    return run_bass_kernel_spmd(nc, in_maps, list(range(N_CORES)), trace=trace)


def kernel(x, Wq, Wk, Wv, Wo, bo, mask, _trace=False, _results=None):
    x = np.asarray(x, dtype=np.float32)
    Wq = np.asarray(Wq, dtype=np.float32)
    Wk = np.asarray(Wk, dtype=np.float32)
    Wv = np.asarray(Wv, dtype=np.float32)
    Wo = np.asarray(Wo, dtype=np.float32)
    bo = np.asarray(bo, dtype=np.float32)
    mask_np = np.asarray(mask).reshape(mask.shape[-2], mask.shape[-1])

    causal = bool(np.array_equal(
        mask_np, np.tril(np.ones((L, L), dtype=bool))))
    if not causal or x.shape != (B, L, D):
        return _numpy_fallback(np.asarray(x), Wq, Wk, Wv, Wo, bo,
                               np.asarray(mask))

    res = run_on_hw(make_in_maps(x, Wq, Wk, Wv, Wo), trace=_trace)
    if _results is not None:
        _results.append(res)
    out = np.empty((B, L, D), dtype=np.float32)
    for b in range(B):
        out[b] = res.results[2 * b]["out"] + res.results[2 * b + 1]["out"] + bo
    return out


# revision 17
# speedup vs baseline: 1.0585x; 1.0585x over previous
"""Causal multi-head attention Trainium2 kernel (8 NeuronCores).

Problem: B=4, L=2048, D=1024, 16 heads x (dh=64, dv=64), causal mask.
Sharding: data-parallel over batch (4) x tensor-parallel over heads (2 groups
of 8). Core c handles batch c//2, head-group c%2. Each core computes its
partial output projection (ctx_g @ Wo_g); the host sums the two head-group
partials per batch and adds the bias.

v4: all-bf16 datapath. x transposed via plain bf16 matmuls against identity
(pipelines ~82ns/tile). S^T = K@Q^T for the two heads of a pair issued
back-to-back into disjoint PE row groups (partitions 0-63/64-127) so they run
concurrently. Attention pipelined at single-k-tile granularity: psc
[128,2,512] (2 PSUM banks) double-buffered decouples the S->exp->PV chain
(v3's 4-bank group was single-buffered and serialized S behind exp). exp is
width-restricted on every diagonal k-tile. Q/K projections for head-pair 0
interleave with the x-transposes; V l-tiles 6-15 and the next pair's Q/K
dribble between attention slots; the output projection for l-chunk j dribbles
into head-pair 3's attention of chunk j+1. Weight DMAs ordered so x loads
first; wo loads during attention.
"""

import numpy as np
from contextlib import ExitStack

import concourse.bass as bass
import concourse.tile as tile
from concourse import bacc, mybir
from concourse.masks import make_identity

F32 = mybir.dt.float32
BF16 = mybir.dt.bfloat16
AF = mybir.ActivationFunctionType

B, L, D = 4, 2048, 1024
N_HEAD, DH, DV = 16, 64, 64
N_CORES = 8
HPC = N_HEAD // 2          # heads per core (8)
OC = HPC * DH              # per-core projection width (512)
NHP = HPC // 2             # head-pairs per core (4)
NCH = L // 512             # q-chunks (4)
NLT = L // 128             # l-tiles (16)


class Emitter:
    """Projection work (optional V l-tile jobs + one head-pair's Q^T/K^T)
    emitted in 8-matmul units so it interleaves with attention."""

    def __init__(self, nc, hp, qkp, wp, psP, xt, wq, wk, vjobs=()):
        self.nc, self.psP, self.xt = nc, psP, xt
        self.units = list(vjobs)           # V jobs first: needed earliest
        self.wq_sb = wp.tile([128, 8, 128], BF16, tag="wq")
        self.wk_sb = wp.tile([128, 8, 128], BF16, tag="wk")
        # weight DMAs ride the scalar engine's queue, parallel to x on sync's
        nc.scalar.dma_start(
            out=self.wq_sb,
            in_=wq[:, hp * 128:(hp + 1) * 128].rearrange("(t p) o -> p t o", p=128))
        nc.scalar.dma_start(
            out=self.wk_sb,
            in_=wk[:, hp * 128:(hp + 1) * 128].rearrange("(t p) o -> p t o", p=128))
        self.qt = qkp.tile([128, L], BF16, tag="qt")
        self.kt = qkp.tile([128, L], BF16, tag="kt")
        for c in range(NCH):               # chunk-major for prologue overlap
            for w_sb, dst in ((self.wq_sb, self.qt), (self.wk_sb, self.kt)):
                self.units.append((w_sb, dst, c))

    def step(self):
        if not self.units:
            return False
        u = self.units.pop(0)
        if callable(u):
            u()
            return True
        w_sb, dst, c = u
        nc = self.nc
        pp = self.psP.tile([128, 512], F32, tag="pp")
        for d in range(8):
            nc.tensor.matmul(pp, w_sb[:, d, :],
                             self.xt[:, d, c * 512:(c + 1) * 512],
                             start=(d == 0), stop=(d == 7))
        nc.vector.tensor_copy(dst[:, c * 512:(c + 1) * 512], pp)
        return True

    def drain(self):
        while self.step():
            pass


def build_nc(l=L):
    assert l % 512 == 0
    nch = l // 512
    nlt = l // 128
    nc = bacc.Bacc("TRN2", target_bir_lowering=False, debug=False,
                   num_devices=N_CORES)

    x = nc.dram_tensor("x", [l, D], BF16, kind="ExternalInput").ap()
    wq = nc.dram_tensor("wq", [D, OC], BF16, kind="ExternalInput").ap()
    wk = nc.dram_tensor("wk", [D, OC], BF16, kind="ExternalInput").ap()
    wv = nc.dram_tensor("wv", [D, OC], BF16, kind="ExternalInput").ap()
    wo = nc.dram_tensor("wo", [OC, D], BF16, kind="ExternalInput").ap()
    out = nc.dram_tensor("out", [l, D], BF16, kind="ExternalOutput").ap()

    with tile.TileContext(nc) as tc, ExitStack() as ctx:
        top = ctx.enter_context(tc.tile_pool(name="top", bufs=1))
        psP = ctx.enter_context(tc.tile_pool(name="psP", bufs=2, space="PSUM"))
        qkp = ctx.enter_context(tc.tile_pool(name="qkp", bufs=2))
        wp = ctx.enter_context(tc.tile_pool(name="wp", bufs=2))

        # V: [128(l), ltile, head, 65] - col 64 is ones (softmax denominator)
        vt = top.tile([128, nlt, HPC, DV + 1], BF16)
        ct = top.tile([128, NHP, l], BF16)        # normalized ctx^T
        xt = top.tile([128, 8, l], BF16)          # x^T, d-major
        wv_sb = top.tile([128, 8, OC], BF16)
        wo_sb = top.tile([128, NHP, D], BF16)
        trilf = top.tile([128, 128], F32)
        tril = top.tile([128, 128], BF16)
        identf = top.tile([128, 128], F32)
        ident = top.tile([128, 128], BF16)
        ones = top.tile([128, 1], BF16)

        # preload the exp activation-table set (~2.7us) during the prologue
        dum = top.tile([1, 8], F32)
        nc.vector.memset(dum, 0.0)
        nc.scalar.activation(dum, dum, AF.Exp)

        make_identity(nc, identf)
        nc.vector.tensor_copy(ident, identf)
        nc.vector.memset(ones, 1.0)
        nc.vector.tensor_copy(
            vt[:, :, :, DV:DV + 1].rearrange("p t h c -> p (t h) c"),
            ones.broadcast_to((128, nlt * HPC, 1)))
        # causal keep-mask for S^T diag blocks: tril[k, q] = 1.0 iff q >= k
        nc.gpsimd.memset(trilf, 0.0)
        nc.gpsimd.affine_select(
            out=trilf, in_=trilf, compare_op=mybir.AluOpType.is_gt,
            fill=1.0, base=0, pattern=[[-1, 128]], channel_multiplier=1)
        nc.vector.tensor_copy(tril, trilf)

        def v_unit(lt):
            pp = psP.tile([128, OC], F32, tag="pp")
            for d in range(8):
                nc.tensor.matmul(pp, xt[:, d, lt * 128:(lt + 1) * 128],
                                 wv_sb[:, d, :], start=(d == 0), stop=(d == 7))
            nc.vector.tensor_copy(
                vt[:, lt, :, 0:DV],
                pp.rearrange("p (h v) -> p h v", h=HPC))

        # ---- Prologue: transpose x || QK(hp0), then wv + V(lt 0-5) --------
        with tc.tile_pool(name="pro", bufs=2) as pro, \
             tc.tile_pool(name="psT", bufs=3, space="PSUM") as psT:
            em = None
            for c in range(nch):
                xst = pro.tile([128, 4, D], BF16, tag="xst")
                nc.sync.dma_start(
                    out=xst,
                    in_=x[c * 512:(c + 1) * 512, :].rearrange(
                        "(s p) d -> p s d", p=128))
                if c == 0:      # x chunk 0 queued first, then wq/wk
                    em = Emitter(nc, 0, qkp, wp, psP, xt, wq, wk)
                if c == 1:      # wv only needed at prologue end (V units)
                    nc.scalar.dma_start(
                        out=wv_sb, in_=wv.rearrange("(t p) o -> p t o", p=128))
                for d in range(8):
                    pt = psT.tile([128, 512], F32, tag="pt")
                    for s in range(4):
                        nc.tensor.matmul(
                            pt[:, s * 128:(s + 1) * 128],
                            xst[:, s, d * 128:(d + 1) * 128], ident,
                            start=True, stop=True)
                    nc.vector.tensor_copy(xt[:, d, c * 512:(c + 1) * 512], pt)
                em.step()       # Q proj chunk c
                em.step()       # K proj chunk c
            for lt in range(6):
                v_unit(lt)

        # ---- Main: attention + dribbled projections -----------------------
        with tc.tile_pool(name="phb", bufs=2) as phb, \
             tc.tile_pool(name="psS", bufs=2, space="PSUM") as psS, \
             tc.tile_pool(name="psC", bufs=1, space="PSUM") as psC:

            def o_unit(lt, n):
                pp = psP.tile([128, 512], F32, tag="pp")
                for v in range(NHP):
                    nc.tensor.matmul(pp, ct[:, v, lt * 128:(lt + 1) * 128],
                                     wo_sb[:, v, n * 512:(n + 1) * 512],
                                     start=(v == 0), stop=(v == NHP - 1))
                ost = ost_tiles[lt % 3]
                nc.vector.tensor_copy(ost[:, n * 512:(n + 1) * 512], pp)
                if n == 1:
                    nc.sync.dma_start(
                        out=out[lt * 128:(lt + 1) * 128, :], in_=ost)

            ost_tiles = [top.tile([128, D], BF16, name=f"ost{i}")
                         for i in range(3)]
            oq = []             # output-projection dribble queue

            for hp in range(NHP):
                qt, kt = em.qt, em.kt
                vjobs = [(lambda lt=lt: v_unit(lt)) for lt in range(6, 16)] \
                    if hp == 0 else ()
                em = (Emitter(nc, hp + 1, qkp, wp, psP, xt, wq, wk, vjobs)
                      if hp + 1 < NHP else None)
                if hp == NHP - 1:   # wo needed from hp3's j0 end onward
                    nc.scalar.dma_start(
                        out=wo_sb, in_=wo.rearrange("(t p) o -> p t o", p=128))
                n_slots = sum(4 * (j + 1) for j in range(nch))
                n_units = len(em.units) if em else 1
                cadence = max(1, n_slots // n_units)
                slot = 0
                for j in range(nch):
                    nkt = 4 * (j + 1)
                    pctx = [psC.tile([DV + 1, 512], F32, tag=f"pctx{h}",
                                     name=f"pctx{h}")
                            for h in range(2)]
                    prevs = []

                    def pv_slot():
                        pkt, ppexp, pc0 = prevs.pop(0)
                        for h in range(2):
                            nc.tensor.matmul(
                                pctx[h][:, pc0:512],
                                vt[:, pkt, 2 * hp + h, :],
                                ppexp[:, h, pc0:512],
                                start=(pkt == 0),
                                stop=(pkt == 4 * j + 3))
                        nonlocal slot
                        if em and slot % cadence == 0:
                            em.step()
                        elif oq:
                            oq.pop(0)()
                        slot += 1

                    for kt_i in range(nkt):
                        c0 = max(0, kt_i - 4 * j) * 128
                        psc = psS.tile([128, 2, 512], F32, tag="psc")
                        for h in range(2):
                            nc.tensor.matmul(
                                psc[:, h, c0:512],
                                kt[64 * h:64 * h + 64,
                                   kt_i * 128:(kt_i + 1) * 128],
                                qt[64 * h:64 * h + 64,
                                   j * 512 + c0:(j + 1) * 512],
                                start=True, stop=True)
                        pexp = phb.tile([128, 2, 512], BF16, tag="pexp",
                                        bufs=4)
                        nc.scalar.activation(pexp[:, :, c0:512],
                                             psc[:, :, c0:512],
                                             AF.Exp, scale=0.125)
                        if kt_i - 4 * j >= 0:       # diag: staircase mask
                            for h in range(2):
                                nc.vector.tensor_mul(
                                    pexp[:, h, c0:c0 + 128],
                                    pexp[:, h, c0:c0 + 128], tril)
                        prevs.append((kt_i, pexp, c0))
                        if len(prevs) > 2:          # PV lags S by 2 k-tiles
                            pv_slot()
                    while prevs:
                        pv_slot()
                    for h in range(2):
                        rs = phb.tile([1, 512], F32, tag="rs")
                        nc.vector.tensor_copy(rs, pctx[h][DV:DV + 1, :])
                        inv = phb.tile([1, 512], F32, tag="inv")
                        nc.vector.reciprocal_approx_fast(out=inv, in_=rs)
                        bc = phb.tile([64, 512], F32, tag="bc")
                        nc.gpsimd.partition_broadcast(out_ap=bc, in_ap=inv)
                        nc.vector.tensor_mul(
                            ct[64 * h:64 * h + 64, hp, j * 512:(j + 1) * 512],
                            pctx[h][0:DV, :], bc)
                    if hp == NHP - 1:   # ct chunk j complete: queue O-proj
                        for lt in range(4 * j, 4 * j + 4):
                            for n in range(2):
                                oq.append(lambda lt=lt, n=n: o_unit(lt, n))
                if em:
                    em.drain()
            while oq:
                oq.pop(0)()

    nc.compile()
    return nc


def make_in_maps(x, Wq, Wk, Wv, Wo):
    import ml_dtypes
    BF = ml_dtypes.bfloat16
    xb = [np.ascontiguousarray(x[b]).astype(BF) for b in range(B)]
    in_maps = []
    for c in range(N_CORES):
        b, g = c // 2, c % 2
        in_maps.append({
            "x": xb[b],
            "wq": np.ascontiguousarray(Wq[:, g * OC:(g + 1) * OC]).astype(BF),
            "wk": np.ascontiguousarray(Wk[:, g * OC:(g + 1) * OC]).astype(BF),
            "wv": np.ascontiguousarray(Wv[:, g * OC:(g + 1) * OC]).astype(BF),
            "wo": np.ascontiguousarray(Wo[g * OC:(g + 1) * OC, :]).astype(BF),
        })
    return in_maps


_NC_CACHE = {}


def _get_nc():
    if "nc" not in _NC_CACHE:
        _NC_CACHE["nc"] = build_nc()
    return _NC_CACHE["nc"]


def _numpy_fallback(x, Wq, Wk, Wv, Wo, bo, mask):
    Bsz, Lq, _ = x.shape
    Q = (x @ Wq).reshape(Bsz, Lq, N_HEAD, DH).transpose(0, 2, 1, 3)
    K = (x @ Wk).reshape(Bsz, Lq, N_HEAD, DH).transpose(0, 2, 1, 3)
    V = (x @ Wv).reshape(Bsz, Lq, N_HEAD, DV).transpose(0, 2, 1, 3)
    s = np.einsum("bhqd,bhkd->bhqk", Q, K) / np.sqrt(np.float32(DH))
    s = np.where(mask, s, -np.inf)
    s = s - s.max(axis=-1, keepdims=True)
    p = np.exp(s)
    p /= p.sum(axis=-1, keepdims=True)
    ctxv = np.einsum("bhqk,bhkv->bhqv", p, V)
    ctxv = ctxv.transpose(0, 2, 1, 3).reshape(Bsz, Lq, N_HEAD * DV)
    return (ctxv @ Wo + bo).astype(np.float32)


def run_on_hw(in_maps, trace=False):
    from concourse.bass_utils import run_bass_kernel_spmd
    nc = _get_nc()
    return run_bass_kernel_spmd(nc, in_maps, list(range(N_CORES)), trace=trace)


def kernel(x, Wq, Wk, Wv, Wo, bo, mask, _trace=False, _results=None):
    x = np.asarray(x, dtype=np.float32)
    Wq = np.asarray(Wq, dtype=np.float32)
    Wk = np.asarray(Wk, dtype=np.float32)
    Wv = np.asarray(Wv, dtype=np.float32)
    Wo = np.asarray(Wo, dtype=np.float32)
    bo = np.asarray(bo, dtype=np.float32)
    mask_np = np.asarray(mask).reshape(mask.shape[-2], mask.shape[-1])

    causal = bool(np.array_equal(
        mask_np, np.tril(np.ones((L, L), dtype=bool))))
    if not causal or x.shape != (B, L, D):
        return _numpy_fallback(np.asarray(x), Wq, Wk, Wv, Wo, bo,
                               np.asarray(mask))

    res = run_on_hw(make_in_maps(x, Wq, Wk, Wv, Wo), trace=_trace)
    if _results is not None:
        _results.append(res)
    out = np.empty((B, L, D), dtype=np.float32)
    for b in range(B):
        out[b] = (res.results[2 * b]["out"].astype(np.float32)
                  + res.results[2 * b + 1]["out"].astype(np.float32) + bo)
    return out
